# revision 12
# baseline (speedup 1.0000x reference)
"""Trainium2 Bass kernel for nn_CustomConvolve (2x2 locally-connected conv).

Reference computation (per image):
  out[w, h] = x[w-1,h-1]*W0(w,h) + x[w-1,h]*W1(w,h)
            + x[w,  h-1]*W2(w,h) + x[w,  h]*W3(w,h) + bias(w,h)
  for w,h in [1, 510]; out row 0 and col 0 are zero.
  Weight index: idx = 511*w + h into weights[261121, 4] / bias[261121].

Sharding: data-parallel over batch. 16 batches / 8 cores = 2 per core;
each core processes 32 (b,c) images of 512x512. weights/bias replicated.

Design (fp16 end-to-end; tolerance is 2e-2, this path measures ~4e-4):
  - Host converts x to fp16 in [group, row, img, col] layout so each
    x-load is 128 partitions x 8KB contiguous descriptors, and
    pre-arranges weights into four "plane" arrays wp[k, w, i] (fp16,
    zero-padded to 512 cols) such that every DVE multiply reads x at
    free offset 0 (4B-aligned, step 1) -> tensor_tensor multiplies run
    in 2x_1P packed mode. The per-tap column shift is absorbed by the
    matmul moving-operand window ([0:510] vs [1:511]).
  - Per output block of P=127 rows: x tile holds rows wo-1..wo+126
    (128 partitions). Taps 0/1 (x row w-1) use partitions 0..126 with an
    identity lhsT; taps 2/3 (x row w) use all 128 partitions with a
    shifted identity (partition p -> psum row p-1). PSUM does all the
    adds (+ bias via a 5th matmul) in fp32 for free, 4 images per PSUM
    tile (4 banks), double-buffered.
  - ScalarE copies PSUM -> fp16 SBUF.
  - DMA engine facts (measured): a DMA that WRITES DRAM drains on a
    single SDMA engine (~19 GB/s) - HWDGE always on the same engine,
    SWDGE rotating per dma_start. Loads with simple 2-dim APs spray
    across all 16 engines by SBUF dest partition. So: loads go on
    sync/HWDGE with 2-dim APs; each group store is split into 4
    partition-range SWDGE dma_starts that land on different engines and
    drain concurrently. Device output is a flat write-order-contiguous
    buffer (8KB descriptors); host reassembles.
  - Output rows 509/510 of all 32 images are packed into one partition
    dim (img,row) block with host-built 0/1 lhsT masks, so the tail
    costs 5 matmuls total instead of 5*32.
"""

import os
import sys

for _p in ("/opt/trn_rl_repo",):
    if _p not in sys.path and os.path.isdir(_p):
        sys.path.append(_p)

import numpy as np

import concourse.bass as bass
import concourse.mybir as mybir
from concourse import bacc
from concourse.bass_utils import run_bass_kernel_spmd
from concourse.tile import TileContext

N_CORES = 8
B, C, W, H = 16, 16, 512, 512
B_PER_CORE = B // N_CORES          # 2
IMGS = B_PER_CORE * C              # 32 images per core
NW = W - 1                         # weight-grid row pitch (511)
NVAL = 510                         # valid output rows/cols: 1..510

BLOCKS = [(1, 127), (128, 127), (255, 127), (382, 127)]  # rows 1..508
N_GROUPS = 4
G = 8                              # images per group (2 PSUM sets of 4)
STORE_SPLITS = [(0, 32), (32, 32), (64, 32), (96, 31)]

F32 = mybir.dt.float32
F16 = mybir.dt.float16

XROW = G * H                       # 4096: one partition's run in x_d
XGRP = W * XROW                    # elements per group in x_d
OROW = G * NVAL                    # 4080: one partition's run in o_d
OBLK = 127 * OROW                  # one (block, group) store
TAIL_OFF = len(BLOCKS) * N_GROUPS * OBLK
O_TOTAL = TAIL_OFF + 64 * NVAL


def _build():
    nc = bacc.Bacc("TRN2", debug=False, target_bir_lowering=False, num_swdge_queues=4)

    x_d = nc.dram_tensor("x", [N_GROUPS, W, G, H], F16, kind="ExternalInput")
    id_d = nc.dram_tensor("ident", [128, 128], F16, kind="ExternalInput")
    xt_d = nc.dram_tensor("xtail", [96, 512], F16, kind="ExternalInput")
    wp_d = nc.dram_tensor("wplanes", [4, 512, 512], F16, kind="ExternalInput")
    bp_d = nc.dram_tensor("bplanes", [512, 512], F16, kind="ExternalInput")
    tm_d = nc.dram_tensor("tmask", [2, 96, 64], F16, kind="ExternalInput")
    tp_d = nc.dram_tensor("tplanes", [4, 96, 512], F16, kind="ExternalInput")
    tb_d = nc.dram_tensor("tbias", [64, 512], F16, kind="ExternalInput")
    o_d = nc.dram_tensor("out", [O_TOTAL], F16, kind="ExternalOutput")

    with TileContext(nc) as tc:
        with (
            tc.tile_pool(name="const", bufs=1) as const_pool,
            tc.tile_pool(name="wpool", bufs=4) as wpool,
            tc.tile_pool(name="xpool", bufs=3) as xpool,
            tc.tile_pool(name="mpool", bufs=2) as mpool,
            tc.tile_pool(name="opool", bufs=4) as opool,
            tc.tile_pool(name="psum", bufs=2, space="PSUM") as psum_pool,
        ):
            ident = const_pool.tile([128, 128], F16)
            nc.gpsimd.dma_start(
                out=ident, in_=bass.AP(id_d, 0, [[128, 128], [1, 128]])
            )

            # Preload ALL blocks' weight planes up front (2.6 MB) so no
            # plane load ever queues behind stores on the gpsimd ring
            # (head-of-line blocking caused 20+ us block-boundary stalls).
            all_planes = []
            for bi, (wo, P) in enumerate(BLOCKS):
                planes = []
                for k in range(4):
                    rows = P if k < 2 else P + 1
                    r0 = wo if k < 2 else wo - 1
                    pl = wpool.tile([rows, 1, 512], F16, tag=f"pl{k}")
                    nc.gpsimd.dma_start(
                        out=pl,
                        in_=bass.AP(
                            wp_d, k * 512 * 512 + r0 * 512, [[512, rows], [1, 512]]
                        ),
                    )
                    planes.append(pl)
                bq = wpool.tile([P, 512], F16, tag="bq")
                nc.gpsimd.dma_start(
                    out=bq, in_=bass.AP(bp_d, wo * 512, [[512, P], [1, 512]])
                )
                all_planes.append((planes, bq))

            for bi, (wo, P) in enumerate(BLOCKS):
                planes, bq = all_planes[bi]

                for g in range(N_GROUPS):
                    # G images' x rows wo-1..wo+P-1, one 8KB-descriptor DMA.
                    x2 = xpool.tile([P + 1, G, H], F16, tag="xt")
                    nc.sync.dma_start(
                        out=x2,
                        in_=bass.AP(
                            x_d,
                            g * XGRP + (wo - 1) * XROW,
                            [[XROW, P + 1], [1, XROW]],
                        ),
                    )

                    # DVE products, all with x at free offset 0 (2x mode).
                    prods = []
                    for k in range(2):  # taps 0/1: x row w-1 (partitions 0..P-1)
                        m = mpool.tile([P, G, 512], F16, tag=f"m{k}")
                        nc.vector.tensor_mul(
                            out=m,
                            in0=x2[0:P],
                            in1=planes[k].to_broadcast((P, G, 512)),
                        )
                        prods.append(m)
                    for k in range(2, 4):  # taps 2/3: x row w (partitions 0..P)
                        m = mpool.tile([P + 1, G, 512], F16, tag=f"m{k}")
                        nc.vector.tensor_mul(
                            out=m,
                            in0=x2,
                            in1=planes[k].to_broadcast((P + 1, G, 512)),
                        )
                        prods.append(m)

                    o2 = opool.tile([P, G, NVAL], F16, tag="ot")
                    lhsT_id = ident[0:P, 0:P]
                    lhsT_sh = ident[0 : P + 1, 1 : P + 1]
                    for half in range(2):
                        j0 = half * 4
                        acc = psum_pool.tile([P, 4, 512], F32, tag="acc")
                        # Group matmuls by lhsT to minimize stationary churn.
                        for j in range(j0, j0 + 4):
                            a = acc[:, j - j0, 0:NVAL]
                            nc.tensor.matmul(
                                a, lhsT_id, prods[0][:, j, 0:NVAL],
                                start=True, stop=False,
                            )
                            nc.tensor.matmul(
                                a, lhsT_id, prods[1][:, j, 1 : NVAL + 1],
                                start=False, stop=False,
                            )
                            nc.tensor.matmul(
                                a, lhsT_id, bq[:, 0:NVAL], start=False, stop=False
                            )
                        for j in range(j0, j0 + 4):
                            a = acc[:, j - j0, 0:NVAL]
                            nc.tensor.matmul(
                                a, lhsT_sh, prods[2][:, j, 0:NVAL],
                                start=False, stop=False,
                            )
                            nc.tensor.matmul(
                                a, lhsT_sh, prods[3][:, j, 1 : NVAL + 1],
                                start=False, stop=True,
                            )
                        nc.scalar.copy(
                            out=o2[:, j0 : j0 + 4, :], in_=acc[:, :, 0:NVAL]
                        )
                        # Partition-range SWDGE stores per half -> rotating
                        # engines, starting as soon as this half is copied.
                        base = (bi * N_GROUPS + g) * OBLK + j0 * NVAL
                        for s0, sn in STORE_SPLITS:
                            nc.gpsimd.dma_start(
                                out=bass.AP(
                                    o_d,
                                    base + s0 * OROW,
                                    [[OROW, sn], [1, 4 * NVAL]],
                                ),
                                in_=o2[s0 : s0 + sn, j0 : j0 + 4, :],
                            )

            # ---- Packed tail: output rows 509/510 for all 32 images ----
            xt = xpool.tile([96, 512], F16, tag="xtail")
            nc.gpsimd.dma_start(out=xt, in_=bass.AP(xt_d, 0, [[512, 96], [1, 512]]))
            tps = []
            for k in range(4):
                tp = wpool.tile([96, 512], F16, tag=f"tp{k}")
                nc.gpsimd.dma_start(
                    out=tp,
                    in_=bass.AP(tp_d, k * 96 * 512, [[512, 96], [1, 512]]),
                )
                tps.append(tp)
            tmu = wpool.tile([96, 64], F16, tag="tmu")
            nc.gpsimd.dma_start(out=tmu, in_=bass.AP(tm_d, 0, [[64, 96], [1, 64]]))
            tmv = wpool.tile([96, 64], F16, tag="tmv")
            nc.gpsimd.dma_start(
                out=tmv, in_=bass.AP(tm_d, 96 * 64, [[64, 96], [1, 64]])
            )
            tb = wpool.tile([64, 512], F16, tag="tb")
            nc.gpsimd.dma_start(out=tb, in_=bass.AP(tb_d, 0, [[512, 64], [1, 512]]))

            mt = []
            for k in range(4):
                m = mpool.tile([96, 512], F16, tag=f"mt{k}")
                nc.vector.tensor_mul(out=m, in0=xt, in1=tps[k])
                mt.append(m)

            acc_t = psum_pool.tile([127, 4, 512], F32, tag="acc")
            at = acc_t[0:64, 0, 0:NVAL]
            nc.tensor.matmul(at, tmu, mt[0][:, 0:NVAL], start=True, stop=False)
            nc.tensor.matmul(at, tmu, mt[1][:, 1 : NVAL + 1], start=False, stop=False)
            nc.tensor.matmul(
                at, ident[0:64, 0:64], tb[:, 0:NVAL], start=False, stop=False
            )
            nc.tensor.matmul(at, tmv, mt[2][:, 0:NVAL], start=False, stop=False)
            nc.tensor.matmul(
                at, tmv, mt[3][:, 1 : NVAL + 1], start=False, stop=True
            )

            ott = opool.tile([64, NVAL], F16, tag="otail")
            nc.scalar.copy(out=ott, in_=acc_t[0:64, 0, 0:NVAL])
            nc.gpsimd.dma_start(
                out=bass.AP(o_d, TAIL_OFF, [[NVAL, 64], [1, NVAL]]),
                in_=ott,
            )

    nc.finalize()
    return nc


_CACHE = {}


def _get_nc():
    if "nc" not in _CACHE:
        _CACHE["nc"] = _build()
    return _CACHE["nc"]


def _host_prep(weights, bias):
    """Build the fp16 plane/mask arrays from the fp32 weights/bias."""
    wg = weights.reshape(NW, NW, 4)  # [w, h, 4] on the 511x511 neuron grid
    bg = bias.reshape(NW, NW)

    wp = np.zeros((4, 512, 512), dtype=np.float16)
    # taps 0/2 multiply x[.., i] (i=h-1, h=i+1): plane[w, i] = Wk(w, i+1)
    wp[0, 1:511, 0:510] = wg[1:511, 1:511, 0].astype(np.float16)
    wp[2, 1:511, 0:510] = wg[1:511, 1:511, 2].astype(np.float16)
    # taps 1/3 multiply x[.., i] (i=h): plane[w, i] = Wk(w, i)
    wp[1, 1:511, 1:511] = wg[1:511, 1:511, 1].astype(np.float16)
    wp[3, 1:511, 1:511] = wg[1:511, 1:511, 3].astype(np.float16)

    bp = np.zeros((512, 512), dtype=np.float16)
    bp[1:511, 0:510] = bg[1:511, 1:511].astype(np.float16)

    # Tail: out rows 509/510, partitions 3j+r <-> (img j, x row 508+r).
    tpl = np.zeros((4, 96, 512), dtype=np.float16)
    tb = np.zeros((64, 512), dtype=np.float16)
    tm = np.zeros((2, 96, 64), dtype=np.float16)
    j = np.arange(32)
    for r in (0, 1):
        w = 509 + r
        for k in (0, 1):  # u-taps at partition 3j+r
            tpl[k, 3 * j + r, :] = wp[k, w, :]
        for k in (2, 3):  # v-taps at partition 3j+1+r
            tpl[k, 3 * j + 1 + r, :] = wp[k, w, :]
        tb[2 * j + r, :] = bp[w, :]
        tm[0, 3 * j + r, 2 * j + r] = 1.0
        tm[1, 3 * j + 1 + r, 2 * j + r] = 1.0
    return wp, bp, tpl, tb, tm


def kernel(x, weights, bias):
    assert x.shape == (B, C, W, H) and x.dtype == np.float32
    nc = _get_nc()

    wp, bp, tpl, tb, tm = _host_prep(weights, bias)
    x16 = x.astype(np.float16).reshape(N_CORES, IMGS, W, H)

    in_maps = []
    for i in range(N_CORES):
        xi = x16[i]
        # [group, row, img-in-group, col] so x-loads are 8KB descriptors.
        xg = np.ascontiguousarray(
            xi.reshape(N_GROUPS, G, W, H).transpose(0, 2, 1, 3)
        )
        # Tail x: partition 3j+r <-> (img j, row 508+r), r in 0..2.
        xtail = np.ascontiguousarray(xi[:, 508:511, :].reshape(96, 512))
        in_maps.append(
            {
                "x": xg,
                "ident": np.eye(128, dtype=np.float16),
                "xtail": xtail,
                "wplanes": wp,
                "bplanes": bp,
                "tmask": tm,
                "tplanes": tpl,
                "tbias": tb,
            }
        )

    trace = os.environ.get("BASS_TRACE") == "1"
    res = run_bass_kernel_spmd(
        nc, in_maps, core_ids=list(range(N_CORES)), trace=trace
    )
    kernel.last_exec_time_ns = res.exec_time_ns
    kernel.last_results = res

    out = np.zeros((B, C, NW, NW), dtype=np.float32)
    for i in range(N_CORES):
        flat = res.results[i]["out"]
        # Main: [block, group, p, img-in-group, h] -> [img, w(1..508), h]
        main = flat[:TAIL_OFF].reshape(len(BLOCKS), N_GROUPS, 127, G, NVAL)
        main = main.transpose(1, 3, 0, 2, 4).reshape(IMGS, 508, NVAL)
        # Tail: [img, r, h] for rows 509/510
        tail = flat[TAIL_OFF:].reshape(IMGS, 2, NVAL)
        core = np.concatenate([main, tail], axis=1)  # [IMGS, 510, 510]
        core = core.reshape(B_PER_CORE, C, NVAL, NVAL)
        out[i * B_PER_CORE : (i + 1) * B_PER_CORE, :, 1:, 1:] = core.astype(
            np.float32
        )
    return out


# revision 13
# speedup vs baseline: 1.0315x; 1.0315x over previous
"""Trainium2 Bass kernel for nn_CustomConvolve (2x2 locally-connected conv).

Reference computation (per image):
  out[w, h] = x[w-1,h-1]*W0(w,h) + x[w-1,h]*W1(w,h)
            + x[w,  h-1]*W2(w,h) + x[w,  h]*W3(w,h) + bias(w,h)
  for w,h in [1, 510]; out row 0 and col 0 are zero.
  Weight index: idx = 511*w + h into weights[261121, 4] / bias[261121].

Sharding: data-parallel over batch. 16 batches / 8 cores = 2 per core;
each core processes 32 (b,c) images of 512x512. weights/bias replicated.

Design (fp16 end-to-end; tolerance is 2e-2, this path measures ~4e-4):
  - Host converts x to fp16 in [group, row, img, col] layout so each
    x-load is 128 partitions x 8KB contiguous descriptors, and
    pre-arranges weights into four "plane" arrays wp[k, w, i] (fp16,
    zero-padded to 512 cols) such that every DVE multiply reads x at
    free offset 0 (4B-aligned, step 1) -> tensor_tensor multiplies run
    in 2x_1P packed mode. The per-tap column shift is absorbed by the
    matmul moving-operand window ([0:510] vs [1:511]).
  - Per output block of P=127 rows: x tile holds rows wo-1..wo+126
    (128 partitions). Taps 0/1 (x row w-1) use partitions 0..126 with an
    identity lhsT; taps 2/3 (x row w) use all 128 partitions with a
    shifted identity (partition p -> psum row p-1). PSUM does all the
    adds (+ bias via a 5th matmul) in fp32 for free, 4 images per PSUM
    tile (4 banks), double-buffered.
  - ScalarE copies PSUM -> fp16 SBUF.
  - DMA engine facts (measured): a DMA that WRITES DRAM drains on a
    single SDMA engine (~19 GB/s) - HWDGE always on the same engine,
    SWDGE rotating per dma_start. Loads with simple 2-dim APs spray
    across all 16 engines by SBUF dest partition. So: loads go on
    sync/HWDGE with 2-dim APs; each group store is split into 4
    partition-range SWDGE dma_starts that land on different engines and
    drain concurrently. Device output is a flat write-order-contiguous
    buffer (8KB descriptors); host reassembles.
  - Output rows 509/510 of all 32 images are packed into one partition
    dim (img,row) block with host-built 0/1 lhsT masks, so the tail
    costs 5 matmuls total instead of 5*32.
"""

import os
import sys

for _p in ("/opt/trn_rl_repo",):
    if _p not in sys.path and os.path.isdir(_p):
        sys.path.append(_p)

import numpy as np

import concourse.bass as bass
import concourse.mybir as mybir
from concourse import bacc
from concourse.bass_utils import run_bass_kernel_spmd
from concourse.tile import TileContext

N_CORES = 8
B, C, W, H = 16, 16, 512, 512
B_PER_CORE = B // N_CORES          # 2
IMGS = B_PER_CORE * C              # 32 images per core
NW = W - 1                         # weight-grid row pitch (511)
NVAL = 510                         # valid output rows/cols: 1..510

BLOCKS = [(1, 127), (128, 127), (255, 127), (382, 127)]  # rows 1..508
N_GROUPS = 4
G = 8                              # images per group (2 PSUM sets of 4)
STORE_SPLITS = [(0, 32), (32, 32), (64, 32), (96, 31)]

F32 = mybir.dt.float32
F16 = mybir.dt.float16

XROW = G * H                       # 4096: one partition's run in x_d
XGRP = W * XROW                    # elements per group in x_d
OROW = G * NVAL                    # 4080: one partition's run in o_d
OBLK = 127 * OROW                  # one (block, group) store
TAIL_OFF = len(BLOCKS) * N_GROUPS * OBLK
O_TOTAL = TAIL_OFF + 64 * NVAL


def _build():
    nc = bacc.Bacc("TRN2", debug=False, target_bir_lowering=False, num_swdge_queues=4)

    x_d = nc.dram_tensor("x", [N_GROUPS, W, G, H], F16, kind="ExternalInput")
    id_d = nc.dram_tensor("ident", [128, 128], F16, kind="ExternalInput")
    xt_d = nc.dram_tensor("xtail", [96, 512], F16, kind="ExternalInput")
    wp_d = nc.dram_tensor("wplanes", [4, 512, 512], F16, kind="ExternalInput")
    bp_d = nc.dram_tensor("bplanes", [512, 512], F16, kind="ExternalInput")
    tm_d = nc.dram_tensor("tmask", [2, 96, 64], F16, kind="ExternalInput")
    tp_d = nc.dram_tensor("tplanes", [4, 96, 512], F16, kind="ExternalInput")
    tb_d = nc.dram_tensor("tbias", [64, 512], F16, kind="ExternalInput")
    o_d = nc.dram_tensor("out", [O_TOTAL], F16, kind="ExternalOutput")

    with TileContext(nc) as tc:
        with (
            tc.tile_pool(name="const", bufs=1) as const_pool,
            tc.tile_pool(name="wpool", bufs=4) as wpool,
            tc.tile_pool(name="xpool", bufs=3) as xpool,
            tc.tile_pool(name="mpool", bufs=2) as mpool,
            tc.tile_pool(name="opool", bufs=4) as opool,
            tc.tile_pool(name="psum", bufs=2, space="PSUM") as psum_pool,
        ):
            ident = const_pool.tile([128, 128], F16)
            nc.gpsimd.dma_start(
                out=ident, in_=bass.AP(id_d, 0, [[128, 128], [1, 128]])
            )

            # Preload ALL blocks' weight planes up front (2.6 MB) so no
            # plane load ever queues behind stores on the gpsimd ring
            # (head-of-line blocking caused 20+ us block-boundary stalls).
            all_planes = []
            for bi, (wo, P) in enumerate(BLOCKS):
                planes = []
                for k in range(4):
                    rows = P if k < 2 else P + 1
                    r0 = wo if k < 2 else wo - 1
                    pl = wpool.tile([rows, 1, 512], F16, tag=f"pl{k}")
                    nc.gpsimd.dma_start(
                        out=pl,
                        in_=bass.AP(
                            wp_d, k * 512 * 512 + r0 * 512, [[512, rows], [1, 512]]
                        ),
                    )
                    planes.append(pl)
                bq = wpool.tile([P, 512], F16, tag="bq")
                nc.gpsimd.dma_start(
                    out=bq, in_=bass.AP(bp_d, wo * 512, [[512, P], [1, 512]])
                )
                all_planes.append((planes, bq))

            for bi, (wo, P) in enumerate(BLOCKS):
                planes, bq = all_planes[bi]

                for g in range(N_GROUPS):
                    # G images' x rows wo-1..wo+P-1, one 8KB-descriptor DMA.
                    x2 = xpool.tile([P + 1, G, H], F16, tag="xt")
                    nc.sync.dma_start(
                        out=x2,
                        in_=bass.AP(
                            x_d,
                            g * XGRP + (wo - 1) * XROW,
                            [[XROW, P + 1], [1, XROW]],
                        ),
                    )

                    # DVE products, all with x at free offset 0 (2x mode).
                    prods = []
                    for k in range(2):  # taps 0/1: x row w-1 (partitions 0..P-1)
                        m = mpool.tile([P, G, 512], F16, tag=f"m{k}")
                        nc.vector.tensor_mul(
                            out=m,
                            in0=x2[0:P],
                            in1=planes[k].to_broadcast((P, G, 512)),
                        )
                        prods.append(m)
                    for k in range(2, 4):  # taps 2/3: x row w (partitions 0..P)
                        m = mpool.tile([P + 1, G, 512], F16, tag=f"m{k}")
                        nc.vector.tensor_mul(
                            out=m,
                            in0=x2,
                            in1=planes[k].to_broadcast((P + 1, G, 512)),
                        )
                        prods.append(m)

                    o2 = opool.tile([P, G, NVAL], F16, tag="ot")
                    lhsT_id = ident[0:P, 0:P]
                    lhsT_sh = ident[0 : P + 1, 1 : P + 1]
                    for half in range(2):
                        j0 = half * 4
                        acc = psum_pool.tile([P, 4, 512], F32, tag="acc")
                        # Group matmuls by lhsT to minimize stationary churn.
                        for j in range(j0, j0 + 4):
                            a = acc[:, j - j0, 0:NVAL]
                            nc.tensor.matmul(
                                a, lhsT_id, prods[0][:, j, 0:NVAL],
                                start=True, stop=False,
                            )
                            nc.tensor.matmul(
                                a, lhsT_id, prods[1][:, j, 1 : NVAL + 1],
                                start=False, stop=False,
                            )
                            nc.tensor.matmul(
                                a, lhsT_id, bq[:, 0:NVAL], start=False, stop=False
                            )
                        for j in range(j0, j0 + 4):
                            a = acc[:, j - j0, 0:NVAL]
                            nc.tensor.matmul(
                                a, lhsT_sh, prods[2][:, j, 0:NVAL],
                                start=False, stop=False,
                            )
                            nc.tensor.matmul(
                                a, lhsT_sh, prods[3][:, j, 1 : NVAL + 1],
                                start=False, stop=True,
                            )
                        nc.scalar.copy(
                            out=o2[:, j0 : j0 + 4, :], in_=acc[:, :, 0:NVAL]
                        )

                    # 4 partition-range SWDGE stores -> 4 rotating engines.
                    base = (bi * N_GROUPS + g) * OBLK
                    for s0, sn in STORE_SPLITS:
                        nc.gpsimd.dma_start(
                            out=bass.AP(
                                o_d,
                                base + s0 * OROW,
                                [[OROW, sn], [1, OROW]],
                            ),
                            in_=o2[s0 : s0 + sn],
                        )

            # ---- Packed tail: output rows 509/510 for all 32 images ----
            xt = xpool.tile([96, 512], F16, tag="xtail")
            nc.gpsimd.dma_start(out=xt, in_=bass.AP(xt_d, 0, [[512, 96], [1, 512]]))
            tps = []
            for k in range(4):
                tp = wpool.tile([96, 512], F16, tag=f"tp{k}")
                nc.gpsimd.dma_start(
                    out=tp,
                    in_=bass.AP(tp_d, k * 96 * 512, [[512, 96], [1, 512]]),
                )
                tps.append(tp)
            tmu = wpool.tile([96, 64], F16, tag="tmu")
            nc.gpsimd.dma_start(out=tmu, in_=bass.AP(tm_d, 0, [[64, 96], [1, 64]]))
            tmv = wpool.tile([96, 64], F16, tag="tmv")
            nc.gpsimd.dma_start(
                out=tmv, in_=bass.AP(tm_d, 96 * 64, [[64, 96], [1, 64]])
            )
            tb = wpool.tile([64, 512], F16, tag="tb")
            nc.gpsimd.dma_start(out=tb, in_=bass.AP(tb_d, 0, [[512, 64], [1, 512]]))

            mt = []
            for k in range(4):
                m = mpool.tile([96, 512], F16, tag=f"mt{k}")
                nc.vector.tensor_mul(out=m, in0=xt, in1=tps[k])
                mt.append(m)

            acc_t = psum_pool.tile([127, 4, 512], F32, tag="acc")
            at = acc_t[0:64, 0, 0:NVAL]
            nc.tensor.matmul(at, tmu, mt[0][:, 0:NVAL], start=True, stop=False)
            nc.tensor.matmul(at, tmu, mt[1][:, 1 : NVAL + 1], start=False, stop=False)
            nc.tensor.matmul(
                at, ident[0:64, 0:64], tb[:, 0:NVAL], start=False, stop=False
            )
            nc.tensor.matmul(at, tmv, mt[2][:, 0:NVAL], start=False, stop=False)
            nc.tensor.matmul(
                at, tmv, mt[3][:, 1 : NVAL + 1], start=False, stop=True
            )

            ott = opool.tile([64, NVAL], F16, tag="otail")
            nc.scalar.copy(out=ott, in_=acc_t[0:64, 0, 0:NVAL])
            nc.gpsimd.dma_start(
                out=bass.AP(o_d, TAIL_OFF, [[NVAL, 64], [1, NVAL]]),
                in_=ott,
            )

    nc.finalize()
    return nc


_CACHE = {}


def _get_nc():
    if "nc" not in _CACHE:
        _CACHE["nc"] = _build()
    return _CACHE["nc"]


def _host_prep(weights, bias):
    """Build the fp16 plane/mask arrays from the fp32 weights/bias."""
    wg = weights.reshape(NW, NW, 4)  # [w, h, 4] on the 511x511 neuron grid
    bg = bias.reshape(NW, NW)

    wp = np.zeros((4, 512, 512), dtype=np.float16)
    # taps 0/2 multiply x[.., i] (i=h-1, h=i+1): plane[w, i] = Wk(w, i+1)
    wp[0, 1:511, 0:510] = wg[1:511, 1:511, 0].astype(np.float16)
    wp[2, 1:511, 0:510] = wg[1:511, 1:511, 2].astype(np.float16)
    # taps 1/3 multiply x[.., i] (i=h): plane[w, i] = Wk(w, i)
    wp[1, 1:511, 1:511] = wg[1:511, 1:511, 1].astype(np.float16)
    wp[3, 1:511, 1:511] = wg[1:511, 1:511, 3].astype(np.float16)

    bp = np.zeros((512, 512), dtype=np.float16)
    bp[1:511, 0:510] = bg[1:511, 1:511].astype(np.float16)

    # Tail: out rows 509/510, partitions 3j+r <-> (img j, x row 508+r).
    tpl = np.zeros((4, 96, 512), dtype=np.float16)
    tb = np.zeros((64, 512), dtype=np.float16)
    tm = np.zeros((2, 96, 64), dtype=np.float16)
    j = np.arange(32)
    for r in (0, 1):
        w = 509 + r
        for k in (0, 1):  # u-taps at partition 3j+r
            tpl[k, 3 * j + r, :] = wp[k, w, :]
        for k in (2, 3):  # v-taps at partition 3j+1+r
            tpl[k, 3 * j + 1 + r, :] = wp[k, w, :]
        tb[2 * j + r, :] = bp[w, :]
        tm[0, 3 * j + r, 2 * j + r] = 1.0
        tm[1, 3 * j + 1 + r, 2 * j + r] = 1.0
    return wp, bp, tpl, tb, tm


def kernel(x, weights, bias):
    assert x.shape == (B, C, W, H) and x.dtype == np.float32
    nc = _get_nc()

    wp, bp, tpl, tb, tm = _host_prep(weights, bias)
    x16 = x.astype(np.float16).reshape(N_CORES, IMGS, W, H)

    in_maps = []
    for i in range(N_CORES):
        xi = x16[i]
        # [group, row, img-in-group, col] so x-loads are 8KB descriptors.
        xg = np.ascontiguousarray(
            xi.reshape(N_GROUPS, G, W, H).transpose(0, 2, 1, 3)
        )
        # Tail x: partition 3j+r <-> (img j, row 508+r), r in 0..2.
        xtail = np.ascontiguousarray(xi[:, 508:511, :].reshape(96, 512))
        in_maps.append(
            {
                "x": xg,
                "ident": np.eye(128, dtype=np.float16),
                "xtail": xtail,
                "wplanes": wp,
                "bplanes": bp,
                "tmask": tm,
                "tplanes": tpl,
                "tbias": tb,
            }
        )

    trace = os.environ.get("BASS_TRACE") == "1"
    res = run_bass_kernel_spmd(
        nc, in_maps, core_ids=list(range(N_CORES)), trace=trace
    )
    kernel.last_exec_time_ns = res.exec_time_ns
    kernel.last_results = res

    out = np.zeros((B, C, NW, NW), dtype=np.float32)
    for i in range(N_CORES):
        flat = res.results[i]["out"]
        # Main: [block, group, p, img-in-group, h] -> [img, w(1..508), h]
        main = flat[:TAIL_OFF].reshape(len(BLOCKS), N_GROUPS, 127, G, NVAL)
        main = main.transpose(1, 3, 0, 2, 4).reshape(IMGS, 508, NVAL)
        # Tail: [img, r, h] for rows 509/510
        tail = flat[TAIL_OFF:].reshape(IMGS, 2, NVAL)
        core = np.concatenate([main, tail], axis=1)  # [IMGS, 510, 510]
        core = core.reshape(B_PER_CORE, C, NVAL, NVAL)
        out[i * B_PER_CORE : (i + 1) * B_PER_CORE, :, 1:, 1:] = core.astype(
            np.float32
        )
    return out
